# revision 37
# baseline (speedup 1.0000x reference)
"""DeepseekOCR text MoE layer on 8 Trainium2 NeuronCores.

Expert-parallel: 4 routed experts per core (bucketed by token count so
every core's slot j has a similar load, exact per-slot capacities);
shared expert sharded 2-way over its intermediate dim x 4-way over
tokens (1408 cols x 512 tokens per core -> no column padding and exact
per-token outputs). Router + token gather/scatter run on host (full-I/O
contract).

Precision split: the routed experts run in fp8 e4m3 with DoubleRow
matmuls (2 MACs/cell/cycle -> ~1.7x PE throughput); the shared expert
stays bf16. The routed contribution is only ~16% of the output L2 norm,
so fp8's ~4%/operand quantization noise lands at ~1.1e-2 end-to-end
(tolerance 2e-2). Weights are pre-scaled by 8 so N(0,0.02) values sit
in e4m3's normal range; the silu input is rescaled by 1/8 on the
activation and the 8*8 down-proj scale is divided out of the host-side
combine weights. All accumulation is fp32 PSUM.

Device program per core:
  routed A (per slot):  hT[h,c] = silu(wg.T @ xT) * (wu.T @ xT)
                        8 DoubleRow MMs per 128-h tile (K=256 each)
  routed B (per slot):  yT[d,c] = wd.T-tiles @ hT  (5 DoubleRow + 1
                        normal MM over the 11 h-tiles)
  shared A: hsT[h,512] over 11 h-tiles;  shared B: ys[512,d] with hsT
            token-tiles stationary (few LDWEIGHTS, exact shapes).
Host: out = scatter_add(yT * combine_w) + pairwise-sum of ys halves.

Engine roles: sync = routed loads, gpsimd = shared loads, scalar =
silu + output stores, vector = psum->sbuf copies, tensor = matmuls.
"""

import numpy as np
import ml_dtypes

import concourse.bacc as bacc
import concourse.mybir as mybir
import concourse.tile as tile
from concourse.bass_utils import run_bass_kernel_spmd

B, S, D = 2, 1024, 2048
E, H, K = 32, 1408, 6
H_SHARED = 2816
ROUTED_SCALE = 1.0
T = B * S                      # 2048 tokens
N_CORES = 8
E_LOC = E // N_CORES           # 4 routed experts per core
NH = H // 128                  # 11 h-tiles per routed expert
ND = D // 512                  # 4 d-groups (512 cols each)
NKD = D // 128                 # 16 contraction k-tiles over D
NSH = H_SHARED // 2 // 128     # 11 shared h-tiles per core (2-way col shard)
TQ = T // 4                    # 512 shared tokens per core (4-way token shard)
NXG = 2                        # xg split into k-chunks for early start
NKP = NKD // 2                 # 8 DoubleRow k-pairs over D
NHP = NH // 2                  # 5 DoubleRow h-pairs (h-tile 10 is the odd one)
WSCALE = 8.0                   # fp8 pre-scale on wg/wu/wd

BF16 = ml_dtypes.bfloat16
E4 = ml_dtypes.float8_e4m3     # TRN FP8_EXP4: max 240, matches ml_dtypes e4m3
f32 = mybir.dt.float32
bf16 = mybir.dt.bfloat16
f8 = mybir.dt.float8e4
DR = mybir.MatmulPerfMode.DoubleRow

LAST_RESULTS = None            # BassKernelResults of the latest run (for test harness)


def _route(x, gate_w):
    """Greedy top-k softmax router, fp32 numpy (matches jax.lax.top_k order)."""
    logits = x @ gate_w.T                              # [T, E]
    m = logits.max(-1, keepdims=True)
    ex = np.exp(logits - m)
    scores = ex / ex.sum(-1, keepdims=True)
    topk_i = np.argsort(-scores, axis=-1, kind="stable")[:, :K]
    topk_w = np.take_along_axis(scores, topk_i, -1) * ROUTED_SCALE
    return topk_i, topk_w.astype(np.float32)


def _build_bass(Cs):
    """Per-core Tile program; Cs[j] = routed token capacity of expert slot j."""
    nc = bacc.Bacc(None, target_bir_lowering=False)

    xg_d = [nc.dram_tensor(f"xg{j}", [NXG, 128, NKD // NXG, Cs[j]], f8,
                           kind="ExternalInput")
            for j in range(E_LOC)]
    wgu = nc.dram_tensor("wgu", [E_LOC, NH, 128, 2, NKD, 128], f8, kind="ExternalInput")
    wdd = nc.dram_tensor("wdd", [E_LOC, ND, 128, 2, NH, 2, 128], f8, kind="ExternalInput")
    xq_t = nc.dram_tensor("xq", [128, NKD, TQ], bf16, kind="ExternalInput")
    sgu = nc.dram_tensor("sgu", [NSH, 128, 2, NKD, 128], bf16, kind="ExternalInput")
    sdd = nc.dram_tensor("sdd", [NSH, 128, ND, 512], bf16, kind="ExternalInput")
    y_d = [nc.dram_tensor(f"y{j}", [128, 4 * ND, Cs[j]], bf16, kind="ExternalOutput")
           for j in range(E_LOC)]
    ys_d = nc.dram_tensor("ys", [TQ, D], bf16, kind="ExternalOutput")

    with tile.TileContext(nc) as tc:
        with (
            tc.tile_pool(name="wgu_p", bufs=11) as wgu_p,
            tc.tile_pool(name="wd_p", bufs=8) as wd_p,
            tc.tile_pool(name="sgu_p", bufs=4) as sgu_p,
            tc.tile_pool(name="sd_p", bufs=1) as sd_p,
            tc.tile_pool(name="xg_p", bufs=2 * NXG) as xg_p,
            tc.tile_pool(name="xq_p", bufs=1) as xq_p,
            tc.tile_pool(name="ht_p", bufs=2) as ht_p,
            tc.tile_pool(name="hst_p", bufs=1) as hst_p,
            tc.tile_pool(name="tmp_p", bufs=2) as tmp_p,
            tc.tile_pool(name="dum_p", bufs=1) as dum_p,
            tc.tile_pool(name="y_p", bufs=3) as y_p,
            tc.tile_pool(name="psA", bufs=4, space="PSUM") as psA,
            tc.tile_pool(name="psB", bufs=4, space="PSUM") as psB,
        ):
            KC = NKD // NXG

            # PE warm-up on zeros while the first loads land (HAM un-throttle)
            warm = tmp_p.tile([128, 512], bf16, tag="tmp")
            nc.vector.memset(warm[:], 0.0)
            pwarm = psA.tile([128, 512], f32, tag="psA")
            for _ in range(8):
                nc.tensor.matmul(pwarm[:], warm[:, :128], warm[:], start=True, stop=True)

            def load_xg(j):
                chunks = []
                for g in range(NXG):
                    xc = xg_p.tile([128, KC, Cs[j]], f8, tag="xg", name=f"xg{j}_{g}")
                    nc.sync.dma_start(xc[:], xg_d[j][g])
                    chunks.append(xc)
                return chunks

            # prologue: only expert-0's inputs on the wire, ordered so the
            # first matmul's operands (h0 slab + xg chunk 0) land first;
            # all of these are contiguous-per-partition (cheap descriptors)
            gu0 = wgu_p.tile([128, 2, NKD, 128], f8, tag="wgu", name="wgu0_h0")
            xc0 = xg_p.tile([128, KC, Cs[0]], f8, tag="xg", name="xg0_0")
            nc.sync.dma_start(xc0[:], xg_d[0][0])
            nc.sync.dma_start(gu0[:], wgu[0, 0])
            xc1 = xg_p.tile([128, KC, Cs[0]], f8, tag="xg", name="xg0_1")
            nc.sync.dma_start(xc1[:], xg_d[0][1])
            xg_next = [xc0, xc1]
            wgu_next = [gu0]
            # expert 0 only: queue ALL remaining wgu slabs now. h1-h6 on
            # sync; h7-h10 on the gpsimd queue, which sits idle until the
            # shared burst fires at h==2 (a single queue tops out around
            # ~200GB/s -> observed ~2us stalls at h=8..10 with sync alone).
            # NEVER put bulk loads on the scalar queue: they head-of-line
            # block the silu stream and stall the PE on PSUM reuse.
            for hn in range(1, NH):
                gu = wgu_p.tile([128, 2, NKD, 128], f8, tag="wgu",
                                name=f"wgu0_h{hn}")
                q = nc.sync if hn < 7 else nc.gpsimd
                q.dma_start(gu[:], wgu[0, hn])
                wgu_next.append(gu)

            hsT = hst_p.tile([128, NSH, TQ], bf16, tag="hst")
            sd_all = sd_p.tile([128, NSH, ND, 512], bf16, tag="sd")
            sgu_slabs = []
            xq = xq_p.tile([128, NKD, TQ], bf16, tag="xq")

            def emit_shared_loads(dep):
                # gated on `dep` (an hT tile written) so this burst cannot
                # compete with expert-0's critical loads. tile_wait_until
                # stops the scheduler hoisting the DMAs above the gating
                # copy; the in-order sequencer + the dum dependency then
                # enforce the delay on hardware. sgu slabs alternate
                # gpsimd/scalar queues (per-queue bandwidth cap); sd slabs
                # queue last on gpsimd -- only needed by shared-B.
                with tc.tile_wait_until(0.02):
                    dum = dum_p.tile([1, 32], bf16, tag="dum")
                    nc.gpsimd.tensor_copy(dum[:], dep)
                    nc.gpsimd.dma_start(xq[:], xq_t[:])
                    for h in range(NSH):
                        s = sgu_p.tile([128, 2, NKD, 128], bf16, tag="sgu",
                                       name=f"sgu{h}")
                        nc.gpsimd.dma_start(s[:], sgu[h])
                        sgu_slabs.append(s)
                    for h in range(NSH):
                        nc.gpsimd.dma_start(sd_all[:, h], sdd[h])

            def shared_a(h):
                """Shared gate/up + silu*mul for one 128-row h-tile."""
                sg = sgu_slabs[h]
                pg = psA.tile([128, TQ], f32, tag="psA")
                for k in range(NKD):
                    nc.tensor.matmul(pg[:], sg[:, 0, k], xq[:, k],
                                     start=(k == 0), stop=(k == NKD - 1))
                pu = psA.tile([128, TQ], f32, tag="psA")
                for k in range(NKD):
                    nc.tensor.matmul(pu[:], sg[:, 1, k], xq[:, k],
                                     start=(k == 0), stop=(k == NKD - 1))
                tmp = tmp_p.tile([128, 512], bf16, tag="tmp")
                nc.scalar.activation(tmp[:], pg[:],
                                     mybir.ActivationFunctionType.Silu)
                nc.vector.tensor_mul(hsT[:, h, :], tmp[:], pu[:])

            # schedule: shared-A h-tiles interleaved between phases A and B.
            # window 0 runs only the two tiles whose sgu slabs load ungated
            # (slab 2+ is pool-gated on sa(0) finishing and would stall).
            # window 3 finishes hsT so shared-B can run BEFORE expert-3's
            # phase B: the kernel then ends on expert-3's small per-dq y
            # stores instead of the bunched 4x512KB ysb drain (~4us saved)
            shared_sched = [[0, 1], [2, 3, 4], [5, 6], [7, 8, 9, 10]]

            def shared_b():
                # stationary = hsT token-tiles (11 LDW per ci)
                for ci in range(TQ // 128):
                    pool, ptag = (psB, "psB") if ci % 2 == 0 else (psA, "psA")
                    pys = [pool.tile([128, 512], f32, tag=ptag,
                                     name=f"pys{ci}_{dg}") for dg in range(ND)]
                    for h in range(NSH):
                        st = hsT[:, h, ci * 128:(ci + 1) * 128]
                        for dg in range(ND):
                            nc.tensor.matmul(pys[dg][:], st, sd_all[:, h, dg],
                                             start=(h == 0), stop=(h == NSH - 1))
                    ysb = y_p.tile([128, 4, 512], bf16, tag="y", name=f"ysb{ci}")
                    for dg in range(ND):
                        nc.vector.tensor_copy(ysb[:, dg, :], pys[dg][:])
                    nc.scalar.dma_start(ys_d[ci * 128:(ci + 1) * 128, :], ysb[:])

            for j in range(E_LOC):
                C = Cs[j]
                NCC = -(-C // 512)
                w_slabs = wgu_next
                xg_chunks = xg_next

                # ---- phase A ----
                hT = ht_p.tile([128, NH, C], f8, tag="ht")
                wd_slabs = {}
                for h in range(NH):
                    # keep a deep DMA lead so the gpsimd shared-weight
                    # burst can't starve the PE (observed 3.4us stall +
                    # HAM re-cool with a 1-slab lead)
                    while len(w_slabs) < min(NH, h + 8):
                        hn = len(w_slabs)
                        gu = wgu_p.tile([128, 2, NKD, 128], f8, tag="wgu",
                                        name=f"wgu{j}_h{hn}")
                        nc.sync.dma_start(gu[:], wgu[j, hn])
                        w_slabs.append(gu)
                    if h == NH - 2:
                        # prefetch ALL 8 wd slabs (2.9MB total) on sync so
                        # phase B never waits on the queue
                        for wq in range(ND):
                            for wh in range(2):
                                wd_s = wd_p.tile([128, NH, 2, 128], f8, tag="wd",
                                                 name=f"wd{j}_{wq}_{wh}")
                                nc.sync.dma_start(wd_s[:], wdd[j, wq, :, wh])
                                wd_slabs[(wq, wh)] = wd_s
                    gu_s = w_slabs[h]
                    for cc in range(NCC):
                        w = min(512, C - cc * 512)
                        cs = slice(cc * 512, cc * 512 + w)
                        # DoubleRow: each MM contracts a pair of 128-row
                        # k-tiles (planes i=0,1 of both operands)
                        pg = psA.tile([128, w], f32, tag="psA")
                        for p in range(NKP):
                            ch = xg_chunks[p // (KC // 2)]
                            lk = 2 * (p % (KC // 2))
                            nc.tensor.matmul(pg[:], gu_s[:, 0, 2 * p:2 * p + 2],
                                             ch[:, lk:lk + 2, cs],
                                             start=(p == 0), stop=(p == NKP - 1),
                                             perf_mode=DR)
                        pu = psA.tile([128, w], f32, tag="psA")
                        for p in range(NKP):
                            ch = xg_chunks[p // (KC // 2)]
                            lk = 2 * (p % (KC // 2))
                            nc.tensor.matmul(pu[:], gu_s[:, 1, 2 * p:2 * p + 2],
                                             ch[:, lk:lk + 2, cs],
                                             start=(p == 0), stop=(p == NKP - 1),
                                             perf_mode=DR)
                        tmp = tmp_p.tile([128, 512], bf16, tag="tmp")
                        # PSUM holds WSCALE*G -> rescale on the silu input
                        nc.scalar.activation(tmp[:, :w], pg[:],
                                             mybir.ActivationFunctionType.Silu,
                                             scale=1.0 / WSCALE)
                        nc.vector.tensor_mul(hT[:, h, cs], tmp[:, :w], pu[:])
                    if j == 0 and h == 2:
                        # h==2 (not 0): give expert-0's wgu slab stream a
                        # head start before the 19MB gpsimd burst begins
                        emit_shared_loads(hT[:1, 0, :32])

                # shared-A tiles here absorb the silu->hT drain latency
                # before phase B's accumulation needs the last h-tile
                for h in shared_sched[j]:
                    shared_a(h)
                if j == E_LOC - 1:
                    shared_b()

                # ---- phase B: stationary = wd d-tiles, moving = hT tokens;
                # 4 d-tiles batched per store (big DMA runs, few descriptors)
                for dq in range(ND):
                    ybuf = y_p.tile([128, 4, C], bf16, tag="y")
                    for i in range(4):
                        dp, dt = i // 2, i % 2
                        if dt == 0:
                            wd_s = wd_slabs[(dq, dp)]
                        for cc in range(NCC):
                            w = min(512, C - cc * 512)
                            cs = slice(cc * 512, cc * 512 + w)
                            py = psB.tile([128, w], f32, tag="psB")
                            # 5 DoubleRow pairs + the lone 11th h-tile
                            for hp in range(NHP):
                                nc.tensor.matmul(py[:], wd_s[:, 2 * hp:2 * hp + 2, dt],
                                                 hT[:, 2 * hp:2 * hp + 2, cs],
                                                 start=(hp == 0), stop=False,
                                                 perf_mode=DR)
                            nc.tensor.matmul(py[:], wd_s[:, NH - 1, dt],
                                             hT[:, NH - 1, cs],
                                             start=False, stop=True)
                            nc.vector.tensor_copy(ybuf[:, i, cs], py[:])
                        if j == E_LOC - 1 and dq == ND - 1 and i == 1:
                            # last expert, last d-group: store the first half
                            # early so the final drain is half-sized
                            nc.scalar.dma_start(
                                y_d[j][:, dq * 4:dq * 4 + 2, :], ybuf[:, :2])
                    if j == E_LOC - 1 and dq == ND - 1:
                        nc.scalar.dma_start(
                            y_d[j][:, dq * 4 + 2:dq * 4 + 4, :], ybuf[:, 2:])
                    else:
                        nc.scalar.dma_start(y_d[j][:, dq * 4:(dq + 1) * 4, :], ybuf[:])

                # next expert's first inputs: emitted after phase B's wd
                # loads so they draw bandwidth from B's window, not A's
                if j + 1 < E_LOC:
                    xg_next = load_xg(j + 1)
                    # head-start the next expert's first 4 wgu slabs on the
                    # (now idle) sync+vector queues while this B drains
                    wgu_next = []
                    for hn in range(4):
                        gu = wgu_p.tile([128, 2, NKD, 128], f8, tag="wgu",
                                        name=f"wgu{j + 1}_h{hn}")
                        nc.sync.dma_start(gu[:], wgu[j + 1, hn])
                        wgu_next.append(gu)

    nc.compile()
    return nc


def kernel(hidden_states, gate_w, wg, wu, wd, swg, swu, swd):
    global LAST_RESULTS
    x = np.ascontiguousarray(np.asarray(hidden_states, np.float32).reshape(T, D))
    gate_w = np.asarray(gate_w, np.float32)
    wg = np.asarray(wg, np.float32)
    wu = np.asarray(wu, np.float32)
    wd = np.asarray(wd, np.float32)
    swg = np.asarray(swg, np.float32)
    swu = np.asarray(swu, np.float32)
    swd = np.asarray(swd, np.float32)

    # ---- host router ----
    topk_i, topk_w = _route(x, gate_w)
    idx = [np.where((topk_i == e).any(-1))[0] for e in range(E)]
    # fold the fp8 weight pre-scale (wg*8 and wd*8 -> y is 64x) out here
    wts = [(topk_w * (topk_i == e))[idx[e]].sum(-1).astype(np.float32)
           / (WSCALE * WSCALE) for e in range(E)]
    cnts = np.array([len(i) for i in idx])
    # bucket experts: slot j on every core serves similarly-loaded experts
    ranked = np.argsort(-cnts, kind="stable")            # expert ids, busiest first
    emap = ranked.reshape(E_LOC, N_CORES)                # emap[j, c] -> expert id
    # capacities rounded to 2 (fp8 [*, 2, C] AP plane stride = C bytes; the
    # ISA step%16 rule applies to the stationary AP whose strides are fixed
    # 128/256B -- the moving plane stride has no such constraint on HW)
    Cs = [max(16, -(-int(cnts[emap[j]].max()) // 2) * 2) for j in range(E_LOC)]

    nc = _build_bass(Cs)

    # ---- host shard + layout prep (all DMA sources partition-major) ----
    xT = np.ascontiguousarray(x.T)                      # [D, T] fp32
    xT16 = xT.astype(BF16)

    in_maps = []
    for c in range(N_CORES):
        hc, tq = c % 2, c // 2
        wgu_np = np.empty((E_LOC, NH, 128, 2, NKD, 128), E4)
        wdd_np = np.empty((E_LOC, ND, 128, 2, NH, 2, 128), E4)
        im = {}
        for j in range(E_LOC):
            e = int(emap[j, c])
            wgu_np[j] = ((np.stack([wg[e], wu[e]]) * WSCALE)
                         .reshape(2, NKD, 128, NH, 128)
                         .transpose(3, 2, 0, 1, 4).astype(E4))
            wdd_np[j] = ((wd[e] * WSCALE).reshape(NH, 128, ND, 2, 2, 128)
                         .transpose(2, 1, 3, 0, 4, 5).astype(E4))
            cnt = int(cnts[e])
            xg_np = np.zeros((NXG, 128, NKD // NXG, Cs[j]), E4)
            xg_np[:, :, :, :cnt] = (xT[:, idx[e]]
                                    .reshape(NXG, NKD // NXG, 128, cnt)
                                    .transpose(0, 2, 1, 3).astype(E4))
            im[f"xg{j}"] = np.ascontiguousarray(xg_np)
        sl = slice(hc * (H_SHARED // 2), (hc + 1) * (H_SHARED // 2))
        sgu_np = (np.stack([swg[:, sl], swu[:, sl]])
                  .reshape(2, NKD, 128, NSH, 128)
                  .transpose(3, 2, 0, 1, 4).astype(BF16))
        sdd_np = swd[sl, :].reshape(NSH, 128, ND, 512).astype(BF16)
        xq_np = (xT16[:, tq * TQ:(tq + 1) * TQ]
                 .reshape(NKD, 128, TQ).transpose(1, 0, 2))
        im.update({
            "wgu": np.ascontiguousarray(wgu_np),
            "wdd": np.ascontiguousarray(wdd_np),
            "xq": np.ascontiguousarray(xq_np),
            "sgu": np.ascontiguousarray(sgu_np),
            "sdd": np.ascontiguousarray(sdd_np),
        })
        in_maps.append(im)

    res = run_bass_kernel_spmd(nc, in_maps, core_ids=list(range(N_CORES)))
    LAST_RESULTS = res

    # ---- host unshard: scatter-add routed outputs, pair-sum shared halves ----
    out = np.zeros((T, D), np.float32)
    for c in range(N_CORES):
        tq = c // 2
        out[tq * TQ:(tq + 1) * TQ] += res.results[c]["ys"].astype(np.float32)
        for j in range(E_LOC):
            e = int(emap[j, c])
            cnt = int(cnts[e])
            y = (res.results[c][f"y{j}"].astype(np.float32)
                 .transpose(1, 0, 2).reshape(D, Cs[j])[:, :cnt])  # [D, cnt]
            out[idx[e]] += (y * wts[e][None, :]).T
    return out.reshape(B, S, D)



# revision 38
# speedup vs baseline: 1.0161x; 1.0161x over previous
"""DeepseekOCR text MoE layer on 8 Trainium2 NeuronCores.

Expert-parallel: 4 routed experts per core (bucketed by token count so
every core's slot j has a similar load, exact per-slot capacities);
shared expert sharded 2-way over its intermediate dim x 4-way over
tokens (1408 cols x 512 tokens per core -> no column padding and exact
per-token outputs). Router + token gather/scatter run on host (full-I/O
contract).

Precision split: the routed experts run in fp8 e4m3 with DoubleRow
matmuls (2 MACs/cell/cycle -> ~1.7x PE throughput); the shared expert
stays bf16. The routed contribution is only ~16% of the output L2 norm,
so fp8's ~4%/operand quantization noise lands at ~1.1e-2 end-to-end
(tolerance 2e-2). Weights are pre-scaled by 8 so N(0,0.02) values sit
in e4m3's normal range; the silu input is rescaled by 1/8 on the
activation and the 8*8 down-proj scale is divided out of the host-side
combine weights. All accumulation is fp32 PSUM.

Device program per core:
  routed A (per slot):  hT[h,c] = silu(wg.T @ xT) * (wu.T @ xT)
                        8 DoubleRow MMs per 128-h tile (K=256 each)
  routed B (per slot):  yT[d,c] = wd.T-tiles @ hT  (5 DoubleRow + 1
                        normal MM over the 11 h-tiles)
  shared A: hsT[h,512] over 11 h-tiles;  shared B: ys[512,d] with hsT
            token-tiles stationary (few LDWEIGHTS, exact shapes).
Host: out = scatter_add(yT * combine_w) + pairwise-sum of ys halves.

Engine roles: sync = routed loads, gpsimd = shared loads, scalar =
silu + output stores, vector = psum->sbuf copies, tensor = matmuls.
"""

import numpy as np
import ml_dtypes

import concourse.bacc as bacc
import concourse.mybir as mybir
import concourse.tile as tile
from concourse.bass_utils import run_bass_kernel_spmd

B, S, D = 2, 1024, 2048
E, H, K = 32, 1408, 6
H_SHARED = 2816
ROUTED_SCALE = 1.0
T = B * S                      # 2048 tokens
N_CORES = 8
E_LOC = E // N_CORES           # 4 routed experts per core
NH = H // 128                  # 11 h-tiles per routed expert
ND = D // 512                  # 4 d-groups (512 cols each)
NKD = D // 128                 # 16 contraction k-tiles over D
NSH = H_SHARED // 2 // 128     # 11 shared h-tiles per core (2-way col shard)
TQ = T // 4                    # 512 shared tokens per core (4-way token shard)
NXG = 2                        # xg split into k-chunks for early start
NKP = NKD // 2                 # 8 DoubleRow k-pairs over D
NHP = NH // 2                  # 5 DoubleRow h-pairs (h-tile 10 is the odd one)
WSCALE = 8.0                   # fp8 pre-scale on wg/wu/wd

BF16 = ml_dtypes.bfloat16
E4 = ml_dtypes.float8_e4m3     # TRN FP8_EXP4: max 240, matches ml_dtypes e4m3
f32 = mybir.dt.float32
bf16 = mybir.dt.bfloat16
f8 = mybir.dt.float8e4
DR = mybir.MatmulPerfMode.DoubleRow

LAST_RESULTS = None            # BassKernelResults of the latest run (for test harness)


def _route(x, gate_w):
    """Greedy top-k softmax router, fp32 numpy (matches jax.lax.top_k order)."""
    logits = x @ gate_w.T                              # [T, E]
    m = logits.max(-1, keepdims=True)
    ex = np.exp(logits - m)
    scores = ex / ex.sum(-1, keepdims=True)
    topk_i = np.argsort(-scores, axis=-1, kind="stable")[:, :K]
    topk_w = np.take_along_axis(scores, topk_i, -1) * ROUTED_SCALE
    return topk_i, topk_w.astype(np.float32)


def _build_bass(Cs):
    """Per-core Tile program; Cs[j] = routed token capacity of expert slot j."""
    nc = bacc.Bacc(None, target_bir_lowering=False)

    xg_d = [nc.dram_tensor(f"xg{j}", [NXG, 128, NKD // NXG, Cs[j]], f8,
                           kind="ExternalInput")
            for j in range(E_LOC)]
    wgu = nc.dram_tensor("wgu", [E_LOC, NH, 128, 2, NKD, 128], f8, kind="ExternalInput")
    wdd = nc.dram_tensor("wdd", [E_LOC, ND, 128, 2, NH, 2, 128], f8, kind="ExternalInput")
    xq_t = nc.dram_tensor("xq", [128, NKD, TQ], bf16, kind="ExternalInput")
    sgu = nc.dram_tensor("sgu", [NSH, 128, 2, NKD, 128], bf16, kind="ExternalInput")
    sdd = nc.dram_tensor("sdd", [NSH, 128, ND, 512], bf16, kind="ExternalInput")
    y_d = [nc.dram_tensor(f"y{j}", [128, 4 * ND, Cs[j]], bf16, kind="ExternalOutput")
           for j in range(E_LOC)]
    ys_d = nc.dram_tensor("ys", [TQ, D], bf16, kind="ExternalOutput")

    with tile.TileContext(nc) as tc:
        with (
            tc.tile_pool(name="wgu_p", bufs=11) as wgu_p,
            tc.tile_pool(name="wd_p", bufs=8) as wd_p,
            tc.tile_pool(name="sgu_p", bufs=4) as sgu_p,
            tc.tile_pool(name="sd_p", bufs=1) as sd_p,
            tc.tile_pool(name="xg_p", bufs=2 * NXG) as xg_p,
            tc.tile_pool(name="xq_p", bufs=1) as xq_p,
            tc.tile_pool(name="ht_p", bufs=2) as ht_p,
            tc.tile_pool(name="hst_p", bufs=1) as hst_p,
            tc.tile_pool(name="tmp_p", bufs=2) as tmp_p,
            tc.tile_pool(name="dum_p", bufs=1) as dum_p,
            tc.tile_pool(name="y_p", bufs=3) as y_p,
            tc.tile_pool(name="psA", bufs=4, space="PSUM") as psA,
            tc.tile_pool(name="psB", bufs=4, space="PSUM") as psB,
        ):
            KC = NKD // NXG

            # PE warm-up on zeros while the first loads land (HAM un-throttle)
            warm = tmp_p.tile([128, 512], bf16, tag="tmp")
            nc.vector.memset(warm[:], 0.0)
            pwarm = psA.tile([128, 512], f32, tag="psA")
            for _ in range(10):
                nc.tensor.matmul(pwarm[:], warm[:, :128], warm[:], start=True, stop=True)

            def load_xg(j):
                chunks = []
                for g in range(NXG):
                    xc = xg_p.tile([128, KC, Cs[j]], f8, tag="xg", name=f"xg{j}_{g}")
                    nc.sync.dma_start(xc[:], xg_d[j][g])
                    chunks.append(xc)
                return chunks

            # prologue: only expert-0's inputs on the wire, ordered so the
            # first matmul's operands (h0 slab + xg chunk 0) land first;
            # all of these are contiguous-per-partition (cheap descriptors)
            gu0 = wgu_p.tile([128, 2, NKD, 128], f8, tag="wgu", name="wgu0_h0")
            xc0 = xg_p.tile([128, KC, Cs[0]], f8, tag="xg", name="xg0_0")
            nc.sync.dma_start(xc0[:], xg_d[0][0])
            nc.sync.dma_start(gu0[:], wgu[0, 0])
            xc1 = xg_p.tile([128, KC, Cs[0]], f8, tag="xg", name="xg0_1")
            nc.sync.dma_start(xc1[:], xg_d[0][1])
            xg_next = [xc0, xc1]
            wgu_next = [gu0]
            # expert 0 only: queue ALL remaining wgu slabs now. h1-h6 on
            # sync; h7-h10 on the gpsimd queue, which sits idle until the
            # shared burst fires at h==2 (a single queue tops out around
            # ~200GB/s -> observed ~2us stalls at h=8..10 with sync alone).
            # NEVER put bulk loads on the scalar queue: they head-of-line
            # block the silu stream and stall the PE on PSUM reuse.
            for hn in range(1, NH):
                gu = wgu_p.tile([128, 2, NKD, 128], f8, tag="wgu",
                                name=f"wgu0_h{hn}")
                q = nc.sync if hn < 7 else nc.gpsimd
                q.dma_start(gu[:], wgu[0, hn])
                wgu_next.append(gu)

            hsT = hst_p.tile([128, NSH, TQ], bf16, tag="hst")
            sd_all = sd_p.tile([128, NSH, ND, 512], bf16, tag="sd")
            sgu_slabs = []
            xq = xq_p.tile([128, NKD, TQ], bf16, tag="xq")

            def emit_shared_loads(dep):
                # gated on `dep` (an hT tile written) so this burst cannot
                # compete with expert-0's critical loads. tile_wait_until
                # stops the scheduler hoisting the DMAs above the gating
                # copy; the in-order sequencer + the dum dependency then
                # enforce the delay on hardware. sgu slabs alternate
                # gpsimd/scalar queues (per-queue bandwidth cap); sd slabs
                # queue last on gpsimd -- only needed by shared-B.
                with tc.tile_wait_until(0.02):
                    dum = dum_p.tile([1, 32], bf16, tag="dum")
                    nc.gpsimd.tensor_copy(dum[:], dep)
                    nc.gpsimd.dma_start(xq[:], xq_t[:])
                    for h in range(NSH):
                        s = sgu_p.tile([128, 2, NKD, 128], bf16, tag="sgu",
                                       name=f"sgu{h}")
                        nc.gpsimd.dma_start(s[:], sgu[h])
                        sgu_slabs.append(s)
                    for h in range(NSH):
                        nc.gpsimd.dma_start(sd_all[:, h], sdd[h])

            def shared_a(h):
                """Shared gate/up + silu*mul for one 128-row h-tile."""
                sg = sgu_slabs[h]
                pg = psA.tile([128, TQ], f32, tag="psA")
                for k in range(NKD):
                    nc.tensor.matmul(pg[:], sg[:, 0, k], xq[:, k],
                                     start=(k == 0), stop=(k == NKD - 1))
                pu = psA.tile([128, TQ], f32, tag="psA")
                for k in range(NKD):
                    nc.tensor.matmul(pu[:], sg[:, 1, k], xq[:, k],
                                     start=(k == 0), stop=(k == NKD - 1))
                tmp = tmp_p.tile([128, 512], bf16, tag="tmp")
                nc.scalar.activation(tmp[:], pg[:],
                                     mybir.ActivationFunctionType.Silu)
                nc.vector.tensor_mul(hsT[:, h, :], tmp[:], pu[:])

            # schedule: shared-A h-tiles interleaved between phases A and B.
            # window 0 runs only the two tiles whose sgu slabs load ungated
            # (slab 2+ is pool-gated on sa(0) finishing and would stall).
            # window 3 finishes hsT so shared-B can run BEFORE expert-3's
            # phase B: the kernel then ends on expert-3's small per-dq y
            # stores instead of the bunched 4x512KB ysb drain (~4us saved)
            shared_sched = [[0, 1], [2, 3, 4], [5, 6], [7, 8, 9, 10]]

            def shared_b():
                # stationary = hsT token-tiles (11 LDW per ci)
                for ci in range(TQ // 128):
                    pool, ptag = (psB, "psB") if ci % 2 == 0 else (psA, "psA")
                    pys = [pool.tile([128, 512], f32, tag=ptag,
                                     name=f"pys{ci}_{dg}") for dg in range(ND)]
                    for h in range(NSH):
                        st = hsT[:, h, ci * 128:(ci + 1) * 128]
                        for dg in range(ND):
                            nc.tensor.matmul(pys[dg][:], st, sd_all[:, h, dg],
                                             start=(h == 0), stop=(h == NSH - 1))
                    ysb = y_p.tile([128, 4, 512], bf16, tag="y", name=f"ysb{ci}")
                    for dg in range(ND):
                        nc.vector.tensor_copy(ysb[:, dg, :], pys[dg][:])
                    nc.scalar.dma_start(ys_d[ci * 128:(ci + 1) * 128, :], ysb[:])

            for j in range(E_LOC):
                C = Cs[j]
                NCC = -(-C // 512)
                w_slabs = wgu_next
                xg_chunks = xg_next

                # ---- phase A ----
                hT = ht_p.tile([128, NH, C], f8, tag="ht")
                wd_slabs = {}
                for h in range(NH):
                    # keep a deep DMA lead so the gpsimd shared-weight
                    # burst can't starve the PE (observed 3.4us stall +
                    # HAM re-cool with a 1-slab lead)
                    while len(w_slabs) < min(NH, h + 8):
                        hn = len(w_slabs)
                        gu = wgu_p.tile([128, 2, NKD, 128], f8, tag="wgu",
                                        name=f"wgu{j}_h{hn}")
                        nc.sync.dma_start(gu[:], wgu[j, hn])
                        w_slabs.append(gu)
                    if h == NH - 2:
                        # prefetch ALL 8 wd slabs (2.9MB total) on sync so
                        # phase B never waits on the queue
                        for wq in range(ND):
                            for wh in range(2):
                                wd_s = wd_p.tile([128, NH, 2, 128], f8, tag="wd",
                                                 name=f"wd{j}_{wq}_{wh}")
                                nc.sync.dma_start(wd_s[:], wdd[j, wq, :, wh])
                                wd_slabs[(wq, wh)] = wd_s
                    gu_s = w_slabs[h]
                    for cc in range(NCC):
                        w = min(512, C - cc * 512)
                        cs = slice(cc * 512, cc * 512 + w)
                        # DoubleRow: each MM contracts a pair of 128-row
                        # k-tiles (planes i=0,1 of both operands)
                        pg = psA.tile([128, w], f32, tag="psA")
                        for p in range(NKP):
                            ch = xg_chunks[p // (KC // 2)]
                            lk = 2 * (p % (KC // 2))
                            nc.tensor.matmul(pg[:], gu_s[:, 0, 2 * p:2 * p + 2],
                                             ch[:, lk:lk + 2, cs],
                                             start=(p == 0), stop=(p == NKP - 1),
                                             perf_mode=DR)
                        pu = psA.tile([128, w], f32, tag="psA")
                        for p in range(NKP):
                            ch = xg_chunks[p // (KC // 2)]
                            lk = 2 * (p % (KC // 2))
                            nc.tensor.matmul(pu[:], gu_s[:, 1, 2 * p:2 * p + 2],
                                             ch[:, lk:lk + 2, cs],
                                             start=(p == 0), stop=(p == NKP - 1),
                                             perf_mode=DR)
                        tmp = tmp_p.tile([128, 512], bf16, tag="tmp")
                        # PSUM holds WSCALE*G -> rescale on the silu input
                        nc.scalar.activation(tmp[:, :w], pg[:],
                                             mybir.ActivationFunctionType.Silu,
                                             scale=1.0 / WSCALE)
                        nc.vector.tensor_mul(hT[:, h, cs], tmp[:, :w], pu[:])
                    if j == 0 and h == 2:
                        # h==2 (not 0): give expert-0's wgu slab stream a
                        # head start before the 19MB gpsimd burst begins
                        emit_shared_loads(hT[:1, 0, :32])

                # shared-A tiles here absorb the silu->hT drain latency
                # before phase B's accumulation needs the last h-tile
                for h in shared_sched[j]:
                    shared_a(h)
                if j == E_LOC - 1:
                    shared_b()

                # ---- phase B: stationary = wd d-tiles, moving = hT tokens;
                # 4 d-tiles batched per store (big DMA runs, few descriptors)
                for dq in range(ND):
                    ybuf = y_p.tile([128, 4, C], bf16, tag="y")
                    for i in range(4):
                        dp, dt = i // 2, i % 2
                        if dt == 0:
                            wd_s = wd_slabs[(dq, dp)]
                        for cc in range(NCC):
                            w = min(512, C - cc * 512)
                            cs = slice(cc * 512, cc * 512 + w)
                            py = psB.tile([128, w], f32, tag="psB")
                            # 5 DoubleRow pairs + the lone 11th h-tile
                            for hp in range(NHP):
                                nc.tensor.matmul(py[:], wd_s[:, 2 * hp:2 * hp + 2, dt],
                                                 hT[:, 2 * hp:2 * hp + 2, cs],
                                                 start=(hp == 0), stop=False,
                                                 perf_mode=DR)
                            nc.tensor.matmul(py[:], wd_s[:, NH - 1, dt],
                                             hT[:, NH - 1, cs],
                                             start=False, stop=True)
                            nc.vector.tensor_copy(ybuf[:, i, cs], py[:])
                        if j == E_LOC - 1 and dq == ND - 1 and i == 1:
                            # last expert, last d-group: store the first half
                            # early so the final drain is half-sized
                            nc.scalar.dma_start(
                                y_d[j][:, dq * 4:dq * 4 + 2, :], ybuf[:, :2])
                    if j == E_LOC - 1 and dq == ND - 1:
                        nc.scalar.dma_start(
                            y_d[j][:, dq * 4 + 2:dq * 4 + 4, :], ybuf[:, 2:])
                    else:
                        nc.scalar.dma_start(y_d[j][:, dq * 4:(dq + 1) * 4, :], ybuf[:])

                # next expert's first inputs: emitted after phase B's wd
                # loads so they draw bandwidth from B's window, not A's
                if j + 1 < E_LOC:
                    xg_next = load_xg(j + 1)
                    # head-start the next expert's first 4 wgu slabs on the
                    # (now idle) sync+vector queues while this B drains
                    wgu_next = []
                    for hn in range(4):
                        gu = wgu_p.tile([128, 2, NKD, 128], f8, tag="wgu",
                                        name=f"wgu{j + 1}_h{hn}")
                        nc.sync.dma_start(gu[:], wgu[j + 1, hn])
                        wgu_next.append(gu)

    nc.compile()
    return nc


def kernel(hidden_states, gate_w, wg, wu, wd, swg, swu, swd):
    global LAST_RESULTS
    x = np.ascontiguousarray(np.asarray(hidden_states, np.float32).reshape(T, D))
    gate_w = np.asarray(gate_w, np.float32)
    wg = np.asarray(wg, np.float32)
    wu = np.asarray(wu, np.float32)
    wd = np.asarray(wd, np.float32)
    swg = np.asarray(swg, np.float32)
    swu = np.asarray(swu, np.float32)
    swd = np.asarray(swd, np.float32)

    # ---- host router ----
    topk_i, topk_w = _route(x, gate_w)
    idx = [np.where((topk_i == e).any(-1))[0] for e in range(E)]
    # fold the fp8 weight pre-scale (wg*8 and wd*8 -> y is 64x) out here
    wts = [(topk_w * (topk_i == e))[idx[e]].sum(-1).astype(np.float32)
           / (WSCALE * WSCALE) for e in range(E)]
    cnts = np.array([len(i) for i in idx])
    # bucket experts: slot j on every core serves similarly-loaded experts
    ranked = np.argsort(-cnts, kind="stable")            # expert ids, busiest first
    emap = ranked.reshape(E_LOC, N_CORES)                # emap[j, c] -> expert id
    # capacities rounded to 2 (fp8 [*, 2, C] AP plane stride = C bytes; the
    # ISA step%16 rule applies to the stationary AP whose strides are fixed
    # 128/256B -- the moving plane stride has no such constraint on HW)
    Cs = [max(16, -(-int(cnts[emap[j]].max()) // 2) * 2) for j in range(E_LOC)]

    nc = _build_bass(Cs)

    # ---- host shard + layout prep (all DMA sources partition-major) ----
    xT = np.ascontiguousarray(x.T)                      # [D, T] fp32
    xT16 = xT.astype(BF16)

    in_maps = []
    for c in range(N_CORES):
        hc, tq = c % 2, c // 2
        wgu_np = np.empty((E_LOC, NH, 128, 2, NKD, 128), E4)
        wdd_np = np.empty((E_LOC, ND, 128, 2, NH, 2, 128), E4)
        im = {}
        for j in range(E_LOC):
            e = int(emap[j, c])
            wgu_np[j] = ((np.stack([wg[e], wu[e]]) * WSCALE)
                         .reshape(2, NKD, 128, NH, 128)
                         .transpose(3, 2, 0, 1, 4).astype(E4))
            wdd_np[j] = ((wd[e] * WSCALE).reshape(NH, 128, ND, 2, 2, 128)
                         .transpose(2, 1, 3, 0, 4, 5).astype(E4))
            cnt = int(cnts[e])
            xg_np = np.zeros((NXG, 128, NKD // NXG, Cs[j]), E4)
            xg_np[:, :, :, :cnt] = (xT[:, idx[e]]
                                    .reshape(NXG, NKD // NXG, 128, cnt)
                                    .transpose(0, 2, 1, 3).astype(E4))
            im[f"xg{j}"] = np.ascontiguousarray(xg_np)
        sl = slice(hc * (H_SHARED // 2), (hc + 1) * (H_SHARED // 2))
        sgu_np = (np.stack([swg[:, sl], swu[:, sl]])
                  .reshape(2, NKD, 128, NSH, 128)
                  .transpose(3, 2, 0, 1, 4).astype(BF16))
        sdd_np = swd[sl, :].reshape(NSH, 128, ND, 512).astype(BF16)
        xq_np = (xT16[:, tq * TQ:(tq + 1) * TQ]
                 .reshape(NKD, 128, TQ).transpose(1, 0, 2))
        im.update({
            "wgu": np.ascontiguousarray(wgu_np),
            "wdd": np.ascontiguousarray(wdd_np),
            "xq": np.ascontiguousarray(xq_np),
            "sgu": np.ascontiguousarray(sgu_np),
            "sdd": np.ascontiguousarray(sdd_np),
        })
        in_maps.append(im)

    res = run_bass_kernel_spmd(nc, in_maps, core_ids=list(range(N_CORES)))
    LAST_RESULTS = res

    # ---- host unshard: scatter-add routed outputs, pair-sum shared halves ----
    out = np.zeros((T, D), np.float32)
    for c in range(N_CORES):
        tq = c // 2
        out[tq * TQ:(tq + 1) * TQ] += res.results[c]["ys"].astype(np.float32)
        for j in range(E_LOC):
            e = int(emap[j, c])
            cnt = int(cnts[e])
            y = (res.results[c][f"y{j}"].astype(np.float32)
                 .transpose(1, 0, 2).reshape(D, Cs[j])[:, :cnt])  # [D, cnt]
            out[idx[e]] += (y * wts[e][None, :]).T
    return out.reshape(B, S, D)

